# revision 23
# baseline (speedup 1.0000x reference)
"""Basket Factorization Machine forward pass on 8 Trainium2 NeuronCores.

y = w_0 + x@w_bias + u.t + t.s + 0.5*(s.s - sq) + u.s   (scalar output)

The computation is sparse: only ~52 rows of the embedding tables matter
(1 user row, 1 target row, 50 basket rows) plus the matching w_bias
entries.  Instead of streaming the 12.8 MB/core b_V shard through the
TensorEngine (the 72 us baseline), each core:

  - streams only its 225 KB x shard to locate its local nonzeros,
  - extracts basket indices ON DEVICE: per 196-wide partition row with
    c <= 2 set bits, (c, sum i, max i) give both indices exactly
    (hi = max, lo = sum - max); compaction to dense gather slots goes
    through a triangular-matmul prefix sum + one-hot matmuls,
  - injects the user/target one-hot indices into the same compaction
    column with iota-weighted matmuls (absent -> 0 -> empty slot),
  - indirect-DMA-gathers the needed rows from a DRAM-resident table
    T = [u_V shard ; b_V shard] augmented with w_bias columns,
  - reduces partial s / sq / bias on the PE into partition 63 and DMAs
    G rows 63..65 (partials + t row + u row) out; the host sums the 8
    per-core partials into the scalar.

Invalid slots encode as out-of-bounds offsets: the gather's bounds
check skips them and the pre-zeroed destination rows contribute 0.

A host-side guard checks the c<=2 assumption (holds for the reference
input distribution at ~50 basket items over 1024 partition buckets per
core); if it ever fails, the original streaming kernel runs instead.
"""

import os
import numpy as np
import ml_dtypes

from concourse import bass, bacc, tile, mybir
from concourse.bass_utils import run_bass_kernel_spmd

_CACHE = {}

N_USR = 500000
N_ITM = 200000
K = 128
M = 8

P = 128
UF = 489
BF = 196
U_SH = P * UF
B_SH = P * BF
U_PAD = M * U_SH
B_PAD = M * B_SH
T_ROWS = U_SH + B_SH
TW = 132
NS = 32
NG = NS + 2

F32 = mybir.dt.float32
I32 = mybir.dt.int32
I8 = mybir.dt.int8
BF16 = mybir.dt.bfloat16


def build_fast3():
    nc = bacc.Bacc(num_devices=M)
    f32 = F32

    xbseg = nc.dram_tensor("xbseg", [P, BF], BF16, kind="ExternalInput")
    xutseg = nc.dram_tensor("xutseg", [P, UF + BF + 2], BF16, kind="ExternalInput")
    rowc = nc.dram_tensor("rowc", [2, UF + BF], f32, kind="ExternalInput")
    tap = nc.dram_tensor("tap", [T_ROWS, TW], f32, kind="ExternalInput")
    out = nc.dram_tensor("out", [NG, TW], f32, kind="ExternalOutput")

    add = mybir.AluOpType.add
    mult = mybir.AluOpType.mult
    sub = mybir.AluOpType.subtract
    mx = mybir.AluOpType.max
    is_ge = mybir.AluOpType.is_ge
    is_lt = mybir.AluOpType.is_lt
    is_eq = mybir.AluOpType.is_equal
    Cp = mybir.ActivationFunctionType.Copy
    X = mybir.AxisListType.X

    with tile.TileContext(nc) as tc:
        with (
            tc.tile_pool(name="io", bufs=1) as io,
            tc.tile_pool(name="ps", bufs=1, space="PSUM") as ps,
        ):
            # ---------- input DMAs on two HWDGE rings ----------
            XSH = io.tile([P, BF], BF16)
            nc.sync.dma_start(XSH[:], xbseg[:])
            XUT = io.tile([P, UF + BF + 2], BF16)
            nc.scalar.dma_start(XUT[:], xutseg[:])
            ROWC = io.tile([2, UF + BF], f32)
            nc.sync.dma_start(ROWC[:], rowc[:])
            XU = XUT[:, 0:UF]
            XT = XUT[:, UF : UF + BF]
            OI2 = XUT[:, UF + BF : UF + BF + 2]  # [ones | iota_p] bf16

            # ---------- device-generated constants (gpsimd, no deps) ------
            IOTWB = io.tile([P, BF], BF16)
            nc.gpsimd.iota(
                IOTWB[:],
                pattern=[[1, BF]],
                base=0,
                channel_multiplier=0,
                allow_small_or_imprecise_dtypes=True,
            )
            IOTW = io.tile([P, NG], f32)
            nc.gpsimd.iota(
                IOTW[:],
                pattern=[[1, NG]],
                base=0,
                channel_multiplier=0,
                allow_small_or_imprecise_dtypes=True,
            )
            IOTWM1 = io.tile([P, NG], f32)
            nc.gpsimd.iota(
                IOTWM1[:],
                pattern=[[1, NG]],
                base=-1,
                channel_multiplier=0,
                allow_small_or_imprecise_dtypes=True,
            )
            OFFGU = io.tile([P, 1], f32)
            nc.gpsimd.iota(
                OFFGU[:],
                pattern=[[0, 1]],
                base=U_SH + 1,
                channel_multiplier=BF,
                allow_small_or_imprecise_dtypes=True,
            )
            ONESB = io.tile([P, P], f32)
            nc.gpsimd.memset(ONESB[:], 1.0)
            LT = io.tile([P, P], f32)
            nc.gpsimd.affine_select(
                LT[:],
                ONESB[:],
                pattern=[[1, P]],
                compare_op=is_ge,
                fill=0.0,
                base=-1,
                channel_multiplier=-1,
            )
            ONES_F = io.tile([P, 1], f32)
            nc.gpsimd.memset(ONES_F[:], 1.0)
            G = io.tile([NG, TW], f32)
            nc.gpsimd.memset(G[:], 0.0)
            TH2 = io.tile([P, 2], f32)
            nc.gpsimd.memset(TH2[:], 0.5)
            nc.gpsimd.memset(TH2[:, 1:2], 1.5)

            # ---------- basket chain ----------
            # C on the scalar engine (junk copy + accumulate)
            CJ = io.tile([P, BF], I8)
            C = io.tile([P, 1], f32)
            nc.scalar.activation(CJ[:], XSH[:], Cp, accum_out=C[:])
            T1 = io.tile([P, BF], BF16)
            I1 = io.tile([P, 1], f32)
            nc.vector.scalar_tensor_tensor(
                T1[:], XSH[:], 1.0, IOTWB[:], op0=mult, op1=mult, accum_out=I1[:]
            )
            HL = io.tile([P, 2], f32)
            nc.vector.tensor_reduce(HL[:, 0:1], T1[:], axis=X, op=mx)
            nc.vector.tensor_tensor(HL[:, 1:2], I1[:], HL[:, 0:1], op=sub)
            BASE_PS = ps.tile([P, 1], f32)
            nc.tensor.matmul(BASE_PS[:], lhsT=LT[:], rhs=C[:], start=True, stop=True)
            BASE = io.tile([P, 1], f32)
            nc.vector.tensor_copy(BASE[:], BASE_PS[:])
            # hi/lo slot values batched: ((HL + OFFGU) * (C >= [0.5,1.5]))
            VAL2 = io.tile([P, 2], f32)
            nc.vector.tensor_tensor(
                VAL2[:], C[:].broadcast_to([P, 2]), TH2[:], op=is_ge
            )
            VAB = io.tile([P, 2], f32)
            nc.vector.scalar_tensor_tensor(
                VAB[:], HL[:], OFFGU[:], VAL2[:], op0=add, op1=mult
            )
            OHVA = io.tile([P, NG], f32)
            nc.vector.tensor_scalar(
                OHVA[:], IOTW[:], BASE[:], VAB[:, 0:1], op0=is_eq, op1=mult
            )
            OHV = io.tile([P, NG], f32)
            nc.vector.tensor_scalar(
                OHV[:], IOTWM1[:], BASE[:], VAB[:, 1:2], op0=is_eq, op1=mult
            )
            # -1/128 per partition sums to exactly -1 per slot in the
            # compaction matmul (slot values < 2^17, so value - k/128 stays
            # exact in f32); empties then read -1 and are bounds-skipped
            nc.vector.scalar_tensor_tensor(
                OHV[:], OHV[:], -0.0078125, OHVA[:], op0=add, op1=add
            )

            # ---------- user/target: one matmul + one stt each ----------
            CSU_PS = ps.tile([2, UF], f32)
            nc.tensor.matmul(CSU_PS[:], lhsT=OI2, rhs=XU, start=True, stop=True)
            CST_PS = ps.tile([2, BF], f32)
            nc.tensor.matmul(CST_PS[:], lhsT=OI2, rhs=XT, start=True, stop=True)
            FF = io.tile([2, 2], f32)
            TMPU = io.tile([2, UF], f32)
            nc.vector.scalar_tensor_tensor(
                TMPU[:],
                CSU_PS[:],
                1.0,
                ROWC[:, 0:UF],
                op0=mult,
                op1=mult,
                accum_out=FF[:, 1:2],
            )
            TMPT = io.tile([2, BF], f32)
            nc.vector.scalar_tensor_tensor(
                TMPT[:],
                CST_PS[:],
                1.0,
                ROWC[:, UF : UF + BF],
                op0=mult,
                op1=mult,
                accum_out=FF[:, 0:1],
            )

            # ---------- compaction: basket one-hots + u/t injection;
            # a constant -1 row folds the offset adjustment into the same
            # PSUM group, so empties read -1 (skipped by the bounds check)
            CP_PS = ps.tile([NG, 1], f32)
            nc.tensor.matmul(
                CP_PS[:], lhsT=OHV[:], rhs=ONES_F[:], start=True, stop=False
            )
            nc.tensor.matmul(
                CP_PS[NS : NS + 2, 0:1],
                lhsT=FF[:],
                rhs=ONES_F[0:2, 0:1],
                start=False,
                stop=True,
                skip_group_check=True,
            )
            API = io.tile([NG, 1], I32)
            nc.vector.tensor_copy(API[:], CP_PS[:])
            nc.gpsimd.indirect_dma_start(
                out=G[:],
                out_offset=None,
                in_=tap[:],
                in_offset=bass.IndirectOffsetOnAxis(ap=API[:], axis=0),
                bounds_check=T_ROWS - 1,
                oob_is_err=False,
            )

            # ---------- ship raw gathered rows; host reduces ----------
            # split across the sync HWDGE ring and the (now idle) SWDGE
            # ring so the two flights overlap
            nc.sync.dma_start(out[0:17, :], G[0:17, :])
            nc.gpsimd.dma_start(out[17:NG, :], G[17:NG, :])

    nc.finalize()
    return nc


def _pad_rows(a, rows):
    if a.shape[0] == rows:
        return a
    pad = np.zeros((rows - a.shape[0],) + a.shape[1:], dtype=a.dtype)
    return np.concatenate([a, pad], axis=0)


_ROWC = None
_OI2 = None


def _make_rowc():
    global _ROWC
    if _ROWC is None:
        rowc = np.zeros((2, UF + BF), np.float32)
        rowc[0, 0:UF] = np.arange(UF, dtype=np.float32) + 1.0
        rowc[0, UF : UF + BF] = np.arange(BF, dtype=np.float32) + (U_SH + 1)
        rowc[1, 0:UF] = float(UF)
        rowc[1, UF : UF + BF] = float(BF)
        _ROWC = rowc
    return _ROWC


def _make_oi2():
    global _OI2
    if _OI2 is None:
        oi2 = np.zeros((P, 2), np.float32)
        oi2[:, 0] = 1.0
        oi2[:, 1] = np.arange(P)
        _OI2 = oi2
    return _OI2


def shard_fast3(x, w_bias, u_V, b_V):
    x = np.asarray(x, np.float32)
    w_bias = np.asarray(w_bias, np.float32).reshape(-1)
    u_V = np.asarray(u_V, np.float32)
    b_V = np.asarray(b_V, np.float32)

    xu = _pad_rows(x[:N_USR], U_PAD).reshape(M, P, UF)
    xt = _pad_rows(x[N_USR : N_USR + N_ITM], B_PAD).reshape(M, P, BF)
    xb = _pad_rows(x[N_USR + N_ITM :], B_PAD).reshape(M, P, BF)
    wbu = _pad_rows(w_bias[:N_USR], U_PAD).reshape(M, U_SH)
    wbt = _pad_rows(w_bias[N_USR : N_USR + N_ITM], B_PAD).reshape(M, B_SH)
    wbb = _pad_rows(w_bias[N_USR + N_ITM :], B_PAD).reshape(M, B_SH)
    uVp = _pad_rows(u_V, U_PAD)
    bVp = _pad_rows(b_V, B_PAD)
    bsq = np.einsum("ij,ij->i", bVp, bVp).reshape(M, B_SH)
    rowc = _make_rowc()

    in_maps = []
    for c in range(M):
        tapc = np.zeros((T_ROWS, TW), np.float32)
        tapc[0:U_SH, 0:K] = uVp[c * U_SH : (c + 1) * U_SH]
        tapc[U_SH:, 0:K] = bVp[c * B_SH : (c + 1) * B_SH]
        tapc[0:U_SH, K] = wbu[c]
        tapc[U_SH:, K] = wbt[c]
        tapc[U_SH:, K + 1] = wbb[c]
        tapc[U_SH:, K + 2] = bsq[c]
        in_maps.append(
            {
                "xbseg": np.ascontiguousarray(xb[c].astype(ml_dtypes.bfloat16)),
                "xutseg": np.ascontiguousarray(
                    np.concatenate([xu[c], xt[c], _make_oi2()], axis=1).astype(
                        ml_dtypes.bfloat16
                    )
                ),
                "rowc": rowc,
                "tap": tapc,
            }
        )
    return in_maps


def combine_fast3(res, w_0):
    pk = np.zeros((NG, TW), np.float64)
    for c in range(M):
        pk += np.asarray(res.results[c]["out"], np.float32)
    s = pk[0:NS, 0:K].sum(axis=0)
    sq = pk[0:NS, K + 2].sum()
    bias = pk[0:NS, K + 1].sum() + pk[NS, K] + pk[NS + 1, K]
    t = pk[NS, 0:K]
    u = pk[NS + 1, 0:K]
    w0v = float(np.asarray(w_0).reshape(-1)[0])
    y = w0v + bias + u @ t + t @ s + 0.5 * (s @ s - sq) + u @ s
    return np.array([[y]], np.float32)


def _fast_guard_ok(x) -> bool:
    """The extraction needs <=2 basket items per (core, partition)
    bucket and one-hot user/target segments with 0/1 values."""
    x = np.asarray(x, np.float32)
    if x.shape[0] < N_USR + 2 * N_ITM:
        return False
    xu = x[:N_USR]
    xt = x[N_USR : N_USR + N_ITM]
    xb = x[N_USR + N_ITM : N_USR + 2 * N_ITM]
    vals = np.unique(x[: N_USR + 2 * N_ITM])
    if not np.all(np.isin(vals, [0.0, 1.0])):
        return False
    if xu.sum() != 1.0 or xt.sum() != 1.0:
        return False
    cnt = _pad_rows(xb, B_PAD).reshape(M * P, BF).sum(axis=1)
    if float(cnt.max()) > 2.0:
        return False
    per_core = cnt.reshape(M, P).sum(axis=1)
    return float(per_core.max()) <= float(NS - 2)


def kernel(**inputs) -> np.ndarray:
    import time as _time

    trace = bool(int(os.environ.get("BFM_TRACE", "0")))
    force = os.environ.get("BFM_FORCE", "")  # "", "fast", "stream"

    use_fast = force != "stream" and (
        force == "fast" or _fast_guard_ok(inputs["x"])
    )

    if use_fast:
        in_maps = shard_fast3(
            inputs["x"], inputs["w_bias"], inputs["u_V"], inputs["b_V"]
        )
        if "fast" not in _CACHE:
            _CACHE["fast"] = build_fast3()
        last_err = None
        for attempt in range(2):
            try:
                res = run_bass_kernel_spmd(
                    _CACHE["fast"], in_maps, core_ids=list(range(M)), trace=trace
                )
                _CACHE["last_result"] = res
                return combine_fast3(res, inputs["w_0"])
            except Exception as e:  # wedged device / runtime fault
                last_err = e
                if attempt == 0:
                    _time.sleep(75)
        if force == "fast":
            raise last_err

    # ---- fallback: stream the full b_V shard (original baseline) ----
    return _kernel_stream(inputs, trace)

# ======================================================================
# Fallback: original streaming kernel (baseline, 72 us) — used only if
# the fast path's sparsity preconditions fail or the device faults.
# ======================================================================

SUPER = 28       # b_V chunks per streaming supertile (196 = 7*28)
N_SUPER = BF // SUPER

OFF_XU = 0
OFF_WBU = UF
OFF_IOT = 2 * UF
OFF_XB = 3 * UF
OFF_XT = 3 * UF + BF
OFF_WBT = 3 * UF + 2 * BF
OFF_WBB = 3 * UF + 3 * BF
OFF_W0 = 3 * UF + 4 * BF
SMF = OFF_W0 + 1  # 2252


def _build_stream():
    nc = bacc.Bacc(num_devices=M)
    f32 = F32

    smalls = nc.dram_tensor("smalls", [P, SMF], f32, kind="ExternalInput")
    xbt2 = nc.dram_tensor("xbt2", [P, BF, 2], f32, kind="ExternalInput")
    uV = nc.dram_tensor("uV", [U_SH, K], f32, kind="ExternalInput")
    bVt = nc.dram_tensor("bVt", [N_SUPER, P, SUPER, K], f32, kind="ExternalInput")
    out = nc.dram_tensor("out", [1, 392], f32, kind="ExternalOutput")

    add = mybir.AluOpType.add
    mult = mybir.AluOpType.mult
    Sq = mybir.ActivationFunctionType.Square
    X = mybir.AxisListType.X

    with tile.TileContext(nc) as tc:
        with (
            tc.tile_pool(name="io", bufs=1) as io,
            tc.tile_pool(name="bstream", bufs=4) as bstream,
            tc.tile_pool(name="scr", bufs=2) as scrpool,
            tc.tile_pool(name="ps", bufs=1, space="PSUM") as ps,
        ):
            bt0 = bstream.tile([P, SUPER, K], f32, tag="bt")
            nc.sync.dma_start(bt0[:], bVt[0])
            LC = io.tile([P, BF, 2], f32)
            nc.sync.dma_start(LC[:], xbt2[:])
            SM = io.tile([P, SMF], f32)
            nc.sync.dma_start(SM[:], smalls[:])
            XU = SM[:, OFF_XU : OFF_XU + UF]
            WU = SM[:, OFF_WBU : OFF_WBU + UF]
            IOTF = SM[:, OFF_IOT : OFF_IOT + UF]
            XB = SM[:, OFF_XB : OFF_XB + BF]
            XT = SM[:, OFF_XT : OFF_XT + BF]
            WT = SM[:, OFF_WBT : OFF_WBT + BF]
            WB = SM[:, OFF_WBB : OFF_WBB + BF]

            ST2 = ps.tile([2, K], f32)
            SQP = [io.tile([P, 1], f32, name=f"sqp{j}") for j in range(N_SUPER)]
            for i in range(N_SUPER):
                if i == 0:
                    bt = bt0
                else:
                    bt = bstream.tile([P, SUPER, K], f32, tag="bt")
                    nc.sync.dma_start(bt[:], bVt[i])
                sqt = scrpool.tile([P, SUPER, K], f32, tag="sqt")
                nc.scalar.activation(sqt[:], bt[:], Sq)
                rns = scrpool.tile([P, SUPER], f32, tag="rns")
                nc.vector.tensor_reduce(rns[:], sqt[:], axis=X, op=add)
                pq = scrpool.tile([P, SUPER], f32, tag="pq")
                nc.vector.tensor_tensor(
                    pq[:], XB[:, i * SUPER : (i + 1) * SUPER], rns[:], op=mult
                )
                q = scrpool.tile([P, 1], f32, tag="q")
                nc.vector.tensor_reduce(q[:], pq[:], axis=X, op=add)
                if i == 0:
                    nc.vector.tensor_copy(SQP[0][:], q[:])
                else:
                    nc.vector.tensor_tensor(SQP[i][:], SQP[i - 1][:], q[:], op=add)
                for c in range(SUPER):
                    t = i * SUPER + c
                    nc.tensor.matmul(
                        ST2[:],
                        lhsT=LC[:, t, :],
                        rhs=bt[:, c, :],
                        start=(t == 0),
                        stop=(t == BF - 1),
                    )

            ACC = io.tile([P, 4], f32)
            nc.vector.memset(ACC[:], 0.0)
            nc.vector.tensor_copy(ACC[:, 3:4], SQP[N_SUPER - 1][:])
            pu = scrpool.tile([P, UF], f32, tag="pu")
            nc.vector.tensor_tensor(pu[:], XU, IOTF, op=mult)
            nc.vector.tensor_reduce(ACC[:, 0:1], pu[:], axis=X, op=add)
            nc.vector.tensor_reduce(ACC[:, 1:2], XU, axis=X, op=add)

            pb = scrpool.tile([P, UF], f32, tag="pu")
            nc.vector.tensor_tensor(pb[:], XU, WU, op=mult)
            B1 = io.tile([P, 1], f32)
            nc.vector.tensor_reduce(B1[:], pb[:], axis=X, op=add)
            pb2 = scrpool.tile([P, BF], f32, tag="pb2")
            nc.vector.tensor_tensor(pb2[:], XT, WT, op=mult)
            B2 = io.tile([P, 1], f32)
            nc.vector.tensor_reduce(B2[:], pb2[:], axis=X, op=add)
            pb3 = scrpool.tile([P, BF], f32, tag="pb2")
            nc.vector.tensor_tensor(pb3[:], XB, WB, op=mult)
            B3 = io.tile([P, 1], f32)
            nc.vector.tensor_reduce(B3[:], pb3[:], axis=X, op=add)
            B12 = io.tile([P, 1], f32)
            nc.vector.tensor_tensor(B12[:], B1[:], B2[:], op=add)
            nc.vector.tensor_tensor(ACC[:, 2:3], B12[:], B3[:], op=add)

            ONES = io.tile([P, 1], f32)
            nc.vector.memset(ONES[:], 1.0)
            RED = ps.tile([1, 4], f32)
            nc.tensor.matmul(RED[:], lhsT=ONES[:], rhs=ACC[:], start=True, stop=True)
            H1 = io.tile([1, 1], f32)
            nc.vector.tensor_copy(H1[:], RED[0:1, 1:2])
            BIAS1 = io.tile([1, 1], f32)
            nc.vector.tensor_copy(BIAS1[:], RED[0:1, 2:3])
            UIDXF = io.tile([1, 2], f32)
            nc.vector.tensor_copy(UIDXF[0:1, 0:1], RED[0:1, 0:1])
            nc.vector.tensor_copy(UIDXF[0:1, 1:2], RED[0:1, 0:1])
            UIDXI = io.tile([1, 2], I32)
            nc.vector.tensor_copy(UIDXI[:], UIDXF[:])

            urow2 = io.tile([2, K], f32)
            nc.vector.memset(urow2[:], 0.0)
            nc.gpsimd.indirect_dma_start(
                out=urow2[:],
                out_offset=None,
                in_=uV[:],
                in_offset=bass.IndirectOffsetOnAxis(ap=UIDXI[:], axis=0),
                bounds_check=U_SH - 1,
                oob_is_err=False,
            )

            PK = io.tile([1, 392], f32)
            nc.vector.memset(PK[:], 0.0)
            STS = io.tile([2, K], f32)
            nc.vector.tensor_copy(STS[:], ST2[:])
            nc.vector.tensor_copy(PK[0:1, 0:K], STS[0:1, 0:K])
            nc.sync.dma_start(PK[0:1, K : 2 * K], STS[1:2, 0:K])
            Hs = io.tile([1, 1], f32)
            nc.vector.tensor_copy(Hs[:], H1[:])
            UH = ps.tile([1, K], f32)
            nc.tensor.matmul(UH[:], lhsT=Hs[:], rhs=urow2[0:1, :], start=True, stop=True)
            nc.vector.tensor_copy(PK[0:1, 2 * K : 3 * K], UH[:])
            nc.vector.tensor_copy(PK[0:1, 384:385], RED[0:1, 3:4])
            nc.vector.tensor_copy(PK[0:1, 385:386], BIAS1[:])
            nc.sync.dma_start(out[:], PK[:])

    nc.finalize()
    return nc


_IOTA = np.arange(U_SH, dtype=np.float32).reshape(P, UF)


def _shard_stream(x, w_bias, u_V, b_V, w_0):
    x = np.asarray(x, np.float32)
    w_bias = np.asarray(w_bias, np.float32).reshape(-1)
    u_V = np.asarray(u_V, np.float32)
    b_V = np.asarray(b_V, np.float32)
    w_0 = np.asarray(w_0, np.float32).reshape(-1)

    xu_full = _pad_rows(x[:N_USR], U_PAD)
    xt_full = _pad_rows(x[N_USR : N_USR + N_ITM], B_PAD)
    xb_full = _pad_rows(x[N_USR + N_ITM : N_USR + 2 * N_ITM], B_PAD)
    wbu_full = _pad_rows(w_bias[:N_USR], U_PAD)
    wbt_full = _pad_rows(w_bias[N_USR : N_USR + N_ITM], B_PAD)
    wbb_full = _pad_rows(w_bias[N_USR + N_ITM : N_USR + 2 * N_ITM], B_PAD)
    uV_full = _pad_rows(u_V, U_PAD)
    bV_full = _pad_rows(b_V, B_PAD)

    def item_layout(v):
        return np.ascontiguousarray(v.reshape(BF, P).T)

    in_maps = []
    for c in range(M):
        us, ue = c * U_SH, (c + 1) * U_SH
        bs, be = c * B_SH, (c + 1) * B_SH
        bshard = bV_full[bs:be]
        bvt = np.ascontiguousarray(
            bshard.reshape(N_SUPER, SUPER, P, K).transpose(0, 2, 1, 3)
        )
        xb_l = item_layout(xb_full[bs:be])
        xt_l = item_layout(xt_full[bs:be])
        sm = np.empty((P, SMF), np.float32)
        sm[:, OFF_XU : OFF_XU + UF] = xu_full[us:ue].reshape(P, UF)
        sm[:, OFF_WBU : OFF_WBU + UF] = wbu_full[us:ue].reshape(P, UF)
        sm[:, OFF_IOT : OFF_IOT + UF] = _IOTA
        sm[:, OFF_XB : OFF_XB + BF] = xb_l
        sm[:, OFF_XT : OFF_XT + BF] = xt_l
        sm[:, OFF_WBT : OFF_WBT + BF] = item_layout(wbt_full[bs:be])
        sm[:, OFF_WBB : OFF_WBB + BF] = item_layout(wbb_full[bs:be])
        sm[:, OFF_W0] = w_0[0]
        in_maps.append(
            {
                "smalls": sm,
                "xbt2": np.ascontiguousarray(np.stack([xb_l, xt_l], axis=-1)),
                "uV": np.ascontiguousarray(uV_full[us:ue]),
                "bVt": bvt,
            }
        )
    return in_maps


def _kernel_stream(inputs, trace):
    in_maps = _shard_stream(
        inputs["x"], inputs["w_bias"], inputs["u_V"], inputs["b_V"], inputs["w_0"]
    )
    if "stream" not in _CACHE:
        _CACHE["stream"] = _build_stream()
    res = run_bass_kernel_spmd(
        _CACHE["stream"], in_maps, core_ids=list(range(M)), trace=trace
    )
    _CACHE["last_result"] = res
    pk = np.zeros(392, np.float64)
    for c in range(M):
        pk += np.asarray(res.results[c]["out"], np.float32).reshape(-1)
    s, t, u = pk[0:K], pk[K : 2 * K], pk[2 * K : 3 * K]
    sq, bias = pk[384], pk[385]
    w0v = float(np.asarray(inputs["w_0"]).reshape(-1)[0])
    y = w0v + bias + u @ t + t @ s + 0.5 * (s @ s - sq) + u @ s
    return np.array([[y]], np.float32)
